# revision 1
# baseline (speedup 1.0000x reference)
"""Trainium2 Bass kernel for nn_CombinedMLPMoEModel (moe_routing).

Strategy (8 NeuronCores, pure data parallel on the batch):
 - Host: shard batch 16384 -> 8 x 2048 tokens, pre-transpose x1/x2/x3 to
   feature-major [Din, tok] so every layer's activation sits with its
   contracted dim on SBUF partitions; replicate weights.
 - On chip, everything stays feature-major: out_fm = W.T @ act_fm with
   lhsT = W exactly as stored [Din, Dout].
 - Precision: the routing decision (top-2 of 8 experts) matches the fp32
   reference only if the MLP chain + router logits carry fp32 accuracy.
   Plain fp32 matmul costs 4 cyc/row on the PE; instead the MLP chain
   uses a 3-term fp32r (tf32-like, 1 cyc/row) split:
       W @ x ~= Whi@xhi + Whi@xlo + Wlo@xhi   (error ~1e-7, 3 cyc/row)
   with Whi/Wlo pre-split on the host and xhi/xlo split on chip.
   Expert matmuls + the folded output layer run in bf16 (routing indices
   stay exact; output rel-err ~3e-3).  LayerNorm statistics use single
   fp32r (their error is a per-token scale/shift, routing-order safe).
 - MoE: dense compute of all 8 experts per 512-token megatile in
   token-major PSUM [128 tok, 512], combined with per-token top-2
   softmax weights via scalar_tensor_tensor, then transposed back to
   feature-major on the PE.
 - The tail (concat(o) @ Wf -> bn -> @ Wr) is linear, so it folds on the
   host into one vector: out = concat(o) @ (Wf @ (scf * Wr)) + c0.
"""

import numpy as np
import ml_dtypes
from contextlib import ExitStack

import concourse.bass as bass
from concourse import bacc
import concourse.mybir as mybir
import concourse.tile as tile
from concourse.bass_utils import run_bass_kernel_spmd

F32 = mybir.dt.float32
F32R = mybir.dt.float32r
BF16 = mybir.dt.bfloat16
AF = mybir.ActivationFunctionType
ALU = mybir.AluOpType
AX = mybir.AxisListType

N_CORES = 8
B = 16384
DIN = 1024
D = 512
H = 1024
D3 = 3 * D          # 1536
E = 8
TOK_CORE = B // N_CORES   # 2048
MT = 512                  # megatile tokens
EPS = 1e-5
NEG_BIG = -1.0e30

_PROGRAM_CACHE = {}


def _chunks(n):
    return n // 128


def build_program(n_tok=TOK_CORE, mt=MT):
    """Build the per-core Bass program (SPMD across the 8 cores)."""
    nc = bacc.Bacc(None, target_bir_lowering=False)
    T = n_tok // mt
    CS = mt
    NC_TOK = _chunks(mt)

    # ---------------- DRAM I/O ----------------
    xs = [nc.dram_tensor(f"x{i+1}t", [DIN, n_tok], F32, kind="ExternalInput")
          for i in range(3)]
    Wp = [nc.dram_tensor(f"Wp{i+1}", [DIN, D], F32, kind="ExternalInput")
          for i in range(3)]
    W1 = nc.dram_tensor("W1", [D3, H], F32, kind="ExternalInput")
    W2 = nc.dram_tensor("W2", [H, H], F32, kind="ExternalInput")
    W3 = nc.dram_tensor("W3", [H, D3], F32, kind="ExternalInput")
    # packed per-feature vectors, [128, chunks] layout
    bp = [nc.dram_tensor(f"bp{i+1}", [128, _chunks(D)], F32, kind="ExternalInput")
          for i in range(3)]
    sc1 = nc.dram_tensor("sc1", [128, _chunks(H)], F32, kind="ExternalInput")
    bi1 = nc.dram_tensor("bi1", [128, _chunks(H)], F32, kind="ExternalInput")
    sc2 = nc.dram_tensor("sc2", [128, _chunks(H)], F32, kind="ExternalInput")
    bi2 = nc.dram_tensor("bi2", [128, _chunks(H)], F32, kind="ExternalInput")
    b3v = nc.dram_tensor("b3v", [128, _chunks(D3)], F32, kind="ExternalInput")
    lngv = nc.dram_tensor("lngv", [128, _chunks(D3)], F32, kind="ExternalInput")
    lnbv = nc.dram_tensor("lnbv", [128, _chunks(D3)], F32, kind="ExternalInput")
    Wg_d = nc.dram_tensor("Wg_r", [128, _chunks(D), E], F32, kind="ExternalInput")
    Wfr_d = nc.dram_tensor("Wfr_r", [128, _chunks(D3), 1], BF16, kind="ExternalInput")
    We_d = nc.dram_tensor("We_r", [128, E, _chunks(D), D], BF16, kind="ExternalInput")
    bexp_d = nc.dram_tensor("bexp_bf", [E, D], BF16, kind="ExternalInput")
    bg_d = nc.dram_tensor("bg_v", [1, E], F32, kind="ExternalInput")
    c0_d = nc.dram_tensor("c0_v", [1, 1], F32, kind="ExternalInput")
    ones_d = nc.dram_tensor("ones_col", [128, 1], F32, kind="ExternalInput")
    onesr_d = nc.dram_tensor("ones_row", [1, 128], F32, kind="ExternalInput")
    ident_d = nc.dram_tensor("ident", [128, 128], F32, kind="ExternalInput")
    out_d = nc.dram_tensor("out", [1, n_tok], F32, kind="ExternalOutput")

    with tile.TileContext(nc) as tc, ExitStack() as ctx:
        cp = ctx.enter_context(tc.tile_pool(name="consts", bufs=1))
        sp = ctx.enter_context(tc.tile_pool(name="work", bufs=1))
        ps = ctx.enter_context(tc.tile_pool(name="psum", bufs=8, space="PSUM"))

        def pt(shape, dtype, tag, bufs=None):
            return sp.tile(shape, dtype, tag=tag, bufs=bufs, name=tag)

        def mmtile(name="p"):
            return ps.tile([128, 512], F32, tag="mm", name=name)

        # ---------------- resident constants ----------------
        We_sb = cp.tile([128, E, _chunks(D), D], BF16, name="We_sb")
        Wg_sb = cp.tile([128, _chunks(D), E], F32, name="Wg_sb")
        nc.sync.dma_start(out=Wg_sb, in_=Wg_d[:, :, :])
        Wfr_sb = cp.tile([128, _chunks(D3), 1], BF16, name="Wfr_sb")
        nc.sync.dma_start(out=Wfr_sb, in_=Wfr_d[:, :, :])
        bexp_sb = cp.tile([E, D], BF16, name="bexp_sb")
        nc.sync.dma_start(out=bexp_sb, in_=bexp_d[:, :])
        ident_sb = cp.tile([128, 128], F32, name="ident_sb")
        nc.sync.dma_start(out=ident_sb, in_=ident_d[:, :])
        ones_r = cp.tile([128, 1], F32R, name="ones_r")
        nc.gpsimd.dma_start(out=ones_r, in_=ones_d[:, :])
        onesr_r = cp.tile([1, 128], F32R, name="onesr_r")
        nc.gpsimd.dma_start(out=onesr_r, in_=onesr_d[:, :])
        bg_bc = cp.tile([128, E], F32, name="bg_bc")
        nc.gpsimd.dma_start(
            out=bg_bc,
            in_=bass.AP(tensor=bg_d[:, :].tensor, offset=0, ap=[[0, 128], [1, E]]),
        )
        c0_sb = cp.tile([1, 1], F32, name="c0_sb")
        nc.sync.dma_start(out=c0_sb, in_=c0_d[:, :])

        def ldvec(dram, nch, name):
            t = cp.tile([128, nch], F32, name=name)
            nc.sync.dma_start(out=t, in_=dram[:, :])
            return t

        bp_sb = [ldvec(bp[i], _chunks(D), f"bp{i}_sb") for i in range(3)]
        sc1_sb = ldvec(sc1, _chunks(H), "sc1_sb")
        bi1_sb = ldvec(bi1, _chunks(H), "bi1_sb")
        sc2_sb = ldvec(sc2, _chunks(H), "sc2_sb")
        bi2_sb = ldvec(bi2, _chunks(H), "bi2_sb")
        b3_sb = ldvec(b3v, _chunks(D3), "b3_sb")
        lng_sb = ldvec(lngv, _chunks(D3), "lng_sb")
        lnb_sb = ldvec(lnbv, _chunks(D3), "lnb_sb")

        def split_act(src_ap):
            """tf32 hi/lo split of one [128, CS] fp32 activation chunk."""
            hi = pt([128, CS], F32R, tag="aph", bufs=2)
            nc.scalar.copy(hi, src_ap)
            lo = pt([128, CS], F32R, tag="apl", bufs=2)
            nc.vector.scalar_tensor_tensor(out=lo, in0=src_ap, scalar=-1.0,
                                           in1=hi.bitcast(F32), op0=ALU.bypass,
                                           op1=ALU.subtract)
            return hi, lo

        def load_w_pair(w_dram, k, dgs, dgw):
            """Load fp32 weight chunk [128, dgw] (k-chunk k, dout slice
            [dgs, dgs+dgw)) and split into tf32 hi/lo on chip."""
            wk = pt([128, 1024], F32, tag="wkf", bufs=3)[:, :dgw]
            nc.sync.dma_start(out=wk,
                              in_=w_dram[128 * k:128 * (k + 1), dgs:dgs + dgw])
            wh = pt([128, 1024], F32R, tag="wkh", bufs=2)[:, :dgw]
            nc.scalar.copy(wh, wk)
            wl = pt([128, 1024], F32R, tag="wkl", bufs=2)[:, :dgw]
            nc.vector.scalar_tensor_tensor(out=wl, in0=wk, scalar=-1.0,
                                           in1=wh.bitcast(F32), op0=ALU.bypass,
                                           op1=ALU.subtract)
            return wh, wl

        def mm3(psum, wh, wl, xh, xl, d, start, stop):
            sl = slice(128 * d, 128 * (d + 1))
            nc.tensor.matmul(psum, wh[:, sl], xh, start=start, stop=False)
            nc.tensor.matmul(psum, wh[:, sl], xl, start=False, stop=False)
            nc.tensor.matmul(psum, wl[:, sl], xh, start=False, stop=stop)

        # ---------------- megatile loop ----------------
        for t in range(T):
            ts = slice(t * CS, (t + 1) * CS)

            # ---- stage A: three projections -> comb [128, 12, CS] ----
            comb = pt([128, _chunks(D3), CS], F32, tag="big12", bufs=2)
            for i in range(3):
                psums = [mmtile(f"pp{i}") for _ in range(4)]
                for k in range(_chunks(DIN)):
                    xk = pt([128, CS], F32, tag="wkf", bufs=3)
                    nc.sync.dma_start(out=xk, in_=xs[i][128 * k:128 * (k + 1), ts])
                    xh, xl = split_act(xk)
                    wh, wl = load_w_pair(Wp[i], k, 0, 512)
                    for d in range(4):
                        mm3(psums[d], wh, wl, xh, xl, d,
                            start=(k == 0), stop=(k == _chunks(DIN) - 1))
                for d in range(4):
                    dd = 4 * i + d
                    nc.scalar.activation(comb[:, dd, :], psums[d], AF.Identity,
                                         bias=bp_sb[i][:, d:d + 1], scale=1.0)

            if t == 0:
                nc.sync.dma_start(out=We_sb, in_=We_d[:, :, :, :])

            # ---- W1 -> h1, W2 -> h2: single dout pass, 8 psum banks ----
            def dense_relu_bn(act_in, w_dram, kch, sc_sb, bi_sb):
                hout = pt([128, _chunks(H), CS], F32, tag="h", bufs=2)
                psums = [mmtile("ph") for _ in range(8)]
                for k in range(kch):
                    ah, al = split_act(act_in[:, k, :])
                    wh, wl = load_w_pair(w_dram, k, 0, 1024)
                    for d in range(8):
                        mm3(psums[d], wh, wl, ah, al, d,
                            start=(k == 0), stop=(k == kch - 1))
                for d in range(8):
                    nc.scalar.activation(hout[:, d, :], psums[d], AF.Relu,
                                         bias=bi_sb[:, d:d + 1],
                                         scale=sc_sb[:, d:d + 1])
                return hout

            h1 = dense_relu_bn(comb, W1, _chunks(D3), sc1_sb, bi1_sb)
            h2 = dense_relu_bn(h1, W2, _chunks(H), sc2_sb, bi2_sb)

            # ---- W3 -> t3 (+b3), two dout groups of 6; fp32r LN stats ----
            t3 = pt([128, _chunks(D3), CS], F32, tag="big12", bufs=2)
            psum_sum = ps.tile([1, CS], F32, tag="mm", name="psum_sum")
            psum_sq = ps.tile([1, CS], F32, tag="mm", name="psum_sq")
            for dg in range(2):
                psums = [mmtile("pw3") for _ in range(6)]
                for k in range(_chunks(H)):
                    ah, al = split_act(h2[:, k, :])
                    wh, wl = load_w_pair(W3, k, 768 * dg, 768)
                    for d in range(6):
                        mm3(psums[d], wh, wl, ah, al, d,
                            start=(k == 0), stop=(k == _chunks(H) - 1))
                for d in range(6):
                    dd = 6 * dg + d
                    nc.scalar.activation(t3[:, dd, :], psums[d], AF.Identity,
                                         bias=b3_sb[:, dd:dd + 1], scale=1.0)
                    t3r = pt([128, CS], F32R, tag="t3r", bufs=2)
                    nc.scalar.copy(t3r, t3[:, dd, :])
                    sqr = pt([128, CS], F32R, tag="sqr", bufs=2)
                    nc.scalar.activation(sqr, t3[:, dd, :], AF.Square)
                    nc.tensor.matmul(psum_sum, ones_r, t3r,
                                     start=(dd == 0), stop=(dd == _chunks(D3) - 1))
                    nc.tensor.matmul(psum_sq, ones_r, sqr,
                                     start=(dd == 0), stop=(dd == _chunks(D3) - 1))

            # ---- LN stats -> r, mean*r; broadcast across partitions ----
            msq = pt([1, CS], F32, tag="st1", bufs=4)
            nc.scalar.activation(msq, psum_sum, AF.Square, scale=1.0 / D3)
            e2p = pt([1, CS], F32, tag="st1", bufs=4)
            nc.scalar.activation(e2p, psum_sq, AF.Copy, bias=EPS, scale=1.0 / D3)
            mean_sb = pt([1, CS], F32, tag="st1", bufs=4)
            nc.scalar.activation(mean_sb, psum_sum, AF.Identity, scale=1.0 / D3)
            veps = pt([1, CS], F32, tag="st1", bufs=4)
            nc.vector.scalar_tensor_tensor(out=veps, in0=msq, scalar=-1.0, in1=e2p,
                                           op0=ALU.mult, op1=ALU.add)
            sdev = pt([1, CS], F32, tag="st1", bufs=4)
            nc.scalar.activation(sdev, veps, AF.Sqrt)
            r_sb = pt([1, CS], F32, tag="st1", bufs=4)
            nc.vector.reciprocal(r_sb, sdev)
            mr_sb = pt([1, CS], F32, tag="st1", bufs=4)
            nc.vector.tensor_mul(mr_sb, mean_sb, r_sb)
            r_r = pt([1, CS], F32R, tag="st1r", bufs=2)
            nc.scalar.copy(r_r, r_sb)
            mr_r = pt([1, CS], F32R, tag="st1r", bufs=2)
            nc.scalar.copy(mr_r, mr_sb)
            psum_rb = mmtile("psum_rb")
            nc.tensor.matmul(psum_rb, onesr_r, r_r, start=True, stop=True)
            rbc = pt([128, CS], F32, tag="bcast", bufs=2)
            nc.scalar.copy(rbc, psum_rb)
            psum_mrb = mmtile("psum_mrb")
            nc.tensor.matmul(psum_mrb, onesr_r, mr_r, start=True, stop=True)
            mrbc = pt([128, CS], F32, tag="bcast", bufs=2)
            nc.scalar.copy(mrbc, psum_mrb)

            # ---- normalize in place: t3 <- LN(t3) =: m ; bf16 copy ----
            m = t3
            for k in range(_chunks(D3)):
                nc.vector.tensor_mul(t3[:, k, :], t3[:, k, :], rbc)
                nc.vector.tensor_sub(t3[:, k, :], t3[:, k, :], mrbc)
                nc.scalar.activation(m[:, k, :], t3[:, k, :], AF.Identity,
                                     bias=lnb_sb[:, k:k + 1], scale=lng_sb[:, k:k + 1])

            # ---- MoE on the three parts + folded output accumulation ----
            outacc = pt([1, CS], F32, tag="outacc", bufs=1)
            for j in range(3):
                o_part = pt([128, 4, CS], BF16, tag="opart", bufs=2)
                mbfp = pt([128, 4, CS], BF16, tag="mbfp", bufs=2)
                for k in range(4):
                    nc.scalar.copy(mbfp[:, k, :], m[:, 4 * j + k, :])
                pend = None
                for c in range(NC_TOK):
                    cs_ = slice(128 * c, 128 * (c + 1))
                    # router logits (fp32, exact routing)
                    psum_log = ps.tile([128, E], F32, tag="mm", name="psum_log")
                    for k in range(4):
                        nc.tensor.matmul(psum_log, m[:, 4 * j + k, cs_],
                                         Wg_sb[:, k, :], start=(k == 0), stop=(k == 3))
                    logits = pt([128, E], F32, tag="logits", bufs=2)
                    nc.vector.tensor_add(logits, psum_log, bg_bc)
                    # top-2 + softmax weights per token
                    max1 = pt([128, 1], F32, tag="max1", bufs=2)
                    nc.vector.reduce_max(max1, logits, axis=AX.X)
                    is1 = pt([128, E], F32, tag="is1", bufs=2)
                    nc.vector.tensor_scalar(out=is1, in0=logits, scalar1=max1,
                                            scalar2=None, op0=ALU.is_equal)
                    l2 = pt([128, E], F32, tag="l2", bufs=2)
                    nc.vector.scalar_tensor_tensor(out=l2, in0=is1, scalar=NEG_BIG,
                                                   in1=logits, op0=ALU.mult,
                                                   op1=ALU.add)
                    max2 = pt([128, 1], F32, tag="max2", bufs=2)
                    nc.vector.reduce_max(max2, l2, axis=AX.X)
                    dlt = pt([128, 1], F32, tag="dlt", bufs=2)
                    nc.vector.tensor_sub(dlt, max1, max2)
                    s1 = pt([128, 1], F32, tag="s1", bufs=2)
                    nc.scalar.activation(s1, dlt, AF.Sigmoid)
                    s2 = pt([128, 1], F32, tag="s2", bufs=2)
                    nc.scalar.activation(s2, dlt, AF.Sigmoid, scale=-1.0)
                    is2 = pt([128, E], F32, tag="is2", bufs=2)
                    nc.vector.tensor_scalar(out=is2, in0=l2, scalar1=max2,
                                            scalar2=None, op0=ALU.is_equal)
                    w_sb = pt([128, E], F32, tag="w_sb", bufs=2)
                    nc.vector.tensor_scalar(out=w_sb, in0=is1, scalar1=s1,
                                            scalar2=None, op0=ALU.mult)
                    nc.vector.scalar_tensor_tensor(out=w_sb, in0=is2, scalar=s2,
                                                   in1=w_sb, op0=ALU.mult, op1=ALU.add)
                    # dense experts in four groups of 2, combined token-major;
                    # o_sb starts from the expert-bias term (w @ bexp)
                    o_sb = pt([128, CS], F32, tag="o_sb", bufs=2)
                    for g in range(4):
                        eps_ = [mmtile("pe") for _ in range(2)]
                        for k in range(4):
                            lhsT = mbfp[:, k, cs_]
                            for ei in range(2):
                                e = 2 * g + ei
                                nc.tensor.matmul(eps_[ei], lhsT, We_sb[:, e, k, :],
                                                 start=(k == 0), stop=(k == 3))
                        if g == 0:
                            # wT + bias matmul while DVE finishes w
                            psum_wt = ps.tile([E, 128], F32, tag="mm", name="psum_wt")
                            nc.tensor.transpose(psum_wt, w_sb, ident_sb)
                            wT_bf = pt([E, 128], BF16, tag="wT_bf", bufs=2)
                            nc.scalar.copy(wT_bf, psum_wt)
                            psum_b = mmtile("psum_b")
                            nc.tensor.matmul(psum_b, wT_bf, bexp_sb,
                                             start=True, stop=True)
                            nc.scalar.copy(o_sb, psum_b)
                        for ei in range(2):
                            e = 2 * g + ei
                            nc.vector.scalar_tensor_tensor(
                                out=o_sb, in0=eps_[ei], scalar=w_sb[:, e:e + 1],
                                in1=o_sb, op0=ALU.mult, op1=ALU.add)
                        if g == 1 and pend is not None:
                            # pipelined: transpose the PREVIOUS chunk's output
                            po, pc = pend
                            psum_ot = mmtile("psum_ot")
                            for d in range(4):
                                nc.tensor.transpose(
                                    psum_ot[:, 128 * d:128 * (d + 1)],
                                    po[:, 128 * d:128 * (d + 1)], ident_sb)
                            nc.scalar.copy(
                                o_part[:, :, slice(128 * pc, 128 * (pc + 1))],
                                psum_ot.rearrange("p (d c) -> p d c", d=4))
                            pend = None
                    pend = (o_sb, c)
                # drain the last chunk's transpose
                po, pc = pend
                psum_ot = mmtile("psum_ot")
                for d in range(4):
                    nc.tensor.transpose(psum_ot[:, 128 * d:128 * (d + 1)],
                                        po[:, 128 * d:128 * (d + 1)], ident_sb)
                nc.scalar.copy(o_part[:, :, slice(128 * pc, 128 * (pc + 1))],
                               psum_ot.rearrange("p (d c) -> p d c", d=4))
                # folded output: outacc += o_part @ Wfr[part j]
                psum_oj = ps.tile([1, CS], F32, tag="mm", name="psum_oj")
                for k in range(4):
                    nc.tensor.matmul(psum_oj, Wfr_sb[:, 4 * j + k, :],
                                     o_part[:, k, :], start=(k == 0), stop=(k == 3))
                if j == 0:
                    nc.scalar.copy(outacc, psum_oj)
                else:
                    nc.vector.tensor_add(outacc, outacc, psum_oj)
            orow = pt([1, CS], F32, tag="orow", bufs=1)
            nc.scalar.activation(orow, outacc, AF.Identity, bias=c0_sb, scale=1.0)
            nc.sync.dma_start(out=out_d[:, ts], in_=orow)

    nc.compile()
    return nc


def _pack_vec(v, nch):
    return np.ascontiguousarray(v.reshape(nch, 128).T.astype(np.float32))


def _tf32_split(w):
    """Split fp32 matrix into tf32-representable hi + lo (RNE at 11
    mantissa bits, matching the PE's fp32r rounding)."""
    w = np.ascontiguousarray(w, np.float32)

    def rnd(x):
        u = x.view(np.uint32)
        keep = ((u + 0x800 + ((u >> 12) & 1)) & 0xFFFFF000).astype(np.uint32)
        return keep.view(np.float32)

    hi = rnd(w)
    lo = rnd((w.astype(np.float64) - hi.astype(np.float64)).astype(np.float32))
    return hi, lo


def prepare_maps(inputs):
    """Host-side sharding + weight prep. Returns per-core input maps."""
    f32 = np.float32
    k64 = 1.0 / np.sqrt(np.float64(1.0) + np.float64(EPS))
    k = f32(k64)
    g1 = inputs["g1"].astype(f32)
    g2 = inputs["g2"].astype(f32)
    # folded output vector: out = concat(o) @ (Wf @ (scf*Wr)) + c0
    scf64 = inputs["bng"].astype(np.float64) * k64
    wfr64 = inputs["Wf"].astype(np.float64) @ (scf64 * inputs["Wr"][:, 0].astype(np.float64))
    c064 = (float(np.dot(inputs["bf"].astype(np.float64) * scf64
                         + inputs["bnb"].astype(np.float64),
                         inputs["Wr"][:, 0].astype(np.float64)))
            + float(inputs["br"][0]))
    consts = {
        "sc1": _pack_vec(g1 * k, _chunks(H)),
        "bi1": _pack_vec(inputs["b1"] * g1 * k + inputs["be1"], _chunks(H)),
        "sc2": _pack_vec(g2 * k, _chunks(H)),
        "bi2": _pack_vec(inputs["b2"] * g2 * k + inputs["be2"], _chunks(H)),
        "b3v": _pack_vec(inputs["b3"], _chunks(D3)),
        "lngv": _pack_vec(inputs["lng"], _chunks(D3)),
        "lnbv": _pack_vec(inputs["lnb"], _chunks(D3)),
        "Wg_r": np.ascontiguousarray(
            inputs["Wg"].reshape(_chunks(D), 128, E).transpose(1, 0, 2), f32),
        "Wfr_r": np.ascontiguousarray(
            wfr64.astype(f32).reshape(_chunks(D3), 128, 1).transpose(1, 0, 2)
        ).astype(ml_dtypes.bfloat16),
        "We_r": np.ascontiguousarray(
            inputs["We"].reshape(E, _chunks(D), 128, D).transpose(2, 0, 1, 3)
        ).astype(ml_dtypes.bfloat16),
        "bexp_bf": np.ascontiguousarray(inputs["bexp"]).astype(ml_dtypes.bfloat16),
        "bg_v": np.ascontiguousarray(inputs["bg"], f32).reshape(1, E),
        "c0_v": np.full((1, 1), c064, f32),
        "ones_col": np.ones((128, 1), f32),
        "ones_row": np.ones((1, 128), f32),
        "ident": np.eye(128, dtype=f32),
    }
    for nm in ["W1", "W2", "W3"]:
        consts[nm] = np.ascontiguousarray(inputs[nm], f32)
    for i in range(3):
        consts[f"Wp{i+1}"] = np.ascontiguousarray(inputs[f"Wp{i+1}"], f32)
        consts[f"bp{i+1}"] = _pack_vec(inputs[f"bp{i+1}"], _chunks(D))
    xts = [np.ascontiguousarray(inputs[f"x{i+1}"].astype(f32).T) for i in range(3)]
    in_maps = []
    for c in range(N_CORES):
        m = dict(consts)
        sl = slice(c * TOK_CORE, (c + 1) * TOK_CORE)
        for i in range(3):
            m[f"x{i+1}t"] = np.ascontiguousarray(xts[i][:, sl])
        in_maps.append(m)
    return in_maps


def run(inputs, trace=False, n_tok=TOK_CORE):
    key = n_tok
    if key not in _PROGRAM_CACHE:
        _PROGRAM_CACHE[key] = build_program(n_tok=n_tok)
    nc = _PROGRAM_CACHE[key]
    in_maps = prepare_maps(inputs)
    res = run_bass_kernel_spmd(nc, in_maps, list(range(N_CORES)), trace=trace)
    rows = [res.results[c]["out"][0] for c in range(N_CORES)]
    out = np.concatenate(rows).reshape(B, 1).astype(np.float32)
    return out, res


def kernel(**inputs):
    out, _ = run(inputs, trace=False)
    return out



# revision 19
# speedup vs baseline: 2.3420x; 2.3420x over previous
"""Trainium2 Bass kernel v2 for nn_CombinedMLPMoEModel (moe_routing).

Strategy (8 NeuronCores, pure data parallel on the batch):
 - Host: shard batch 16384 -> 8 x 2048 tokens, pre-transpose x1/x2/x3 to
   feature-major [Din, tok]; replicate (folded) weights.
 - Algebraic folds done on the host in fp64:
     * proj+W1:  h1pre = x @ (Wp_i @ W1_i) summed over i  (MAC-neutral,
       removes the intermediate `comb` activation entirely)
     * the tail (concat(o) @ Wf -> bn -> @ Wr) is linear in the MoE
       outputs o_j, so each expert's whole [tok,512] output folds to a
       scalar: S_j[:,e] = m_j @ (We_e @ wfr_j) + bexp_e . wfr_j  where
       wfr = Wf @ (bng*k*Wr).  The dense-expert compute, output
       transposes, bias matmuls and final fold all collapse into a tiny
       [512->8] fp32 matmul per part.
     * LayerNorm folds into those router/S matmuls: with
       A_j=(W3_j*lng_j)@Wg, B_j=(W3_j*lng_j)@V_j computed from h2
       directly, logits/S are affine in (r, mu*r): the normalized m is
       never materialized, and W3's matmul error never touches routing.
 - Precision: chain matmuls run fp32r (1 cyc/row).  TERMS controls a
   hi/lo activation/weight split per stage (1=single pass, 2=act split,
   3=act+weight split ~= exact) to keep top-2 routing flips rare; the
   router/S matmuls and LN-stat affine math are fp32-exact given h2.
 - Routing: top-2 of 8 via masked reduce_max, softmax pair via Sigmoid,
   combined with the folded per-expert scalars in one
   tensor_tensor_reduce per part.
"""

import numpy as np
from contextlib import ExitStack

import concourse.bass as bass
from concourse import bacc
import concourse.mybir as mybir
import concourse.tile as tile
from concourse.bass_utils import run_bass_kernel_spmd

F32 = mybir.dt.float32
F32R = mybir.dt.float32r
AF = mybir.ActivationFunctionType
ALU = mybir.AluOpType
AX = mybir.AxisListType

N_CORES = 8
B = 16384
DIN = 1024
D = 512
H = 1024
D3 = 3 * D          # 1536
E = 8
TOK_CORE = B // N_CORES   # 2048
MT = 512                  # megatile tokens
T = TOK_CORE // MT        # 4
EPS = 1e-5
NEG_BIG = -1.0e30

# per-stage fp32r term count: (stage1 x@Wc, W2, W3)
TERMS = (2, 2, 1)

_PROGRAM_CACHE = {}


def build_program(terms=TERMS, n_tok=TOK_CORE, use_ttr=False, skip_tail=False):
    nc = bacc.Bacc(None, target_bir_lowering=False)
    nT = n_tok // MT

    # ---------------- DRAM I/O ----------------
    xs = [nc.dram_tensor(f"x{i+1}t", [DIN, n_tok], F32, kind="ExternalInput")
          for i in range(3)]
    Wc = [nc.dram_tensor(f"Wc{i+1}", [DIN, H], F32, kind="ExternalInput")
          for i in range(3)]
    W2d = nc.dram_tensor("W2f", [H, H], F32, kind="ExternalInput")
    W3d = nc.dram_tensor("W3f", [H, D3], F32, kind="ExternalInput")
    sc1d = nc.dram_tensor("sc1", [128, 8], F32, kind="ExternalInput")
    bi1d = nc.dram_tensor("bi1", [128, 8], F32, kind="ExternalInput")
    sc2d = nc.dram_tensor("sc2", [128, 8], F32, kind="ExternalInput")
    bi2d = nc.dram_tensor("bi2", [128, 8], F32, kind="ExternalInput")
    b3d = nc.dram_tensor("b3v", [128, 12], F32, kind="ExternalInput")
    ABd = nc.dram_tensor("ABr", [128, 8, 48], F32, kind="ExternalInput")
    CAd = nc.dram_tensor("CAv", [1, 48], F32, kind="ExternalInput")
    Dnd = nc.dram_tensor("Dnv", [1, 48], F32, kind="ExternalInput")
    Ebd = nc.dram_tensor("Ebv", [1, 48], F32, kind="ExternalInput")
    c0d = nc.dram_tensor("c0v", [1, 1], F32, kind="ExternalInput")
    onesd = nc.dram_tensor("ones_col", [128, 1], F32, kind="ExternalInput")
    outd = nc.dram_tensor("out", [128, 4 * nT], F32, kind="ExternalOutput")

    with tile.TileContext(nc) as tc, ExitStack() as ctx:
        cp = ctx.enter_context(tc.tile_pool(name="consts", bufs=1))
        sp = ctx.enter_context(tc.tile_pool(name="work", bufs=1))
        ps = ctx.enter_context(tc.tile_pool(name="psum", bufs=8, space="PSUM"))

        def pt(shape, dtype, tag, bufs=2, space=None):
            return sp.tile(shape, dtype, tag=tag, bufs=bufs, name=tag,
                           space=space)

        # ---------------- resident constants ----------------
        AB_sb = cp.tile([128, 8, 48], F32, name="AB_sb")
        nc.sync.dma_start(out=AB_sb, in_=ABd[:, :, :])
        sc1_sb = cp.tile([128, 8], F32, name="sc1_sb")
        nc.sync.dma_start(out=sc1_sb, in_=sc1d[:, :])
        bi1_sb = cp.tile([128, 8], F32, name="bi1_sb")
        nc.sync.dma_start(out=bi1_sb, in_=bi1d[:, :])
        sc2_sb = cp.tile([128, 8], F32, name="sc2_sb")
        nc.sync.dma_start(out=sc2_sb, in_=sc2d[:, :])
        bi2_sb = cp.tile([128, 8], F32, name="bi2_sb")
        nc.sync.dma_start(out=bi2_sb, in_=bi2d[:, :])
        b3_sb = cp.tile([128, 12], F32, name="b3_sb")
        nc.sync.dma_start(out=b3_sb, in_=b3d[:, :])
        ones_r = cp.tile([128, 1], F32R, name="ones_r")
        nc.gpsimd.dma_start(out=ones_r, in_=onesd[:, :])

        def bcast48(dram, name):
            t = cp.tile([128, 48], F32, name=name)
            nc.gpsimd.dma_start(
                out=t,
                in_=bass.AP(tensor=dram[:, :].tensor, offset=0,
                            ap=[[0, 128], [1, 48]]))
            return t

        CA_bc = bcast48(CAd, "CA_bc")
        Dn_bc = bcast48(Dnd, "Dn_bc")
        Eb_bc = bcast48(Ebd, "Eb_bc")
        c0_bc = cp.tile([128, 1], F32, name="c0_bc")
        nc.gpsimd.dma_start(
            out=c0_bc,
            in_=bass.AP(tensor=c0d[:, :].tensor, offset=0, ap=[[0, 128], [1, 1]]))
        out_tm = cp.tile([128, 4 * nT], F32, name="out_tm")

        def split_act(src, nterms):
            """Return list of fp32r act term APs for one [128, 512] chunk.
            The BIR verifier requires fp32r matmul operands to come from a
            rounding producer, so even 1-term acts go through a copy."""
            ah = pt([128, MT], F32R, tag="ah", bufs=3)
            nc.scalar.copy(ah, src)
            if nterms == 1:
                return [ah]
            al = pt([128, MT], F32R, tag="al", bufs=3)
            nc.vector.scalar_tensor_tensor(out=al, in0=src, scalar=-1.0,
                                           in1=ah.bitcast(F32), op0=ALU.bypass,
                                           op1=ALU.subtract)
            return [ah, al]

        def load_w(dram, k, cs, cw, nterms):
            """Load weight chunk [128, cw] (k-chunk k, col slice cs..cs+cw);
            return list of fp32r weight term APs."""
            wkf = pt([128, 1024], F32, tag="wkf", bufs=3)[:, :cw]
            nc.sync.dma_start(out=wkf,
                              in_=dram[128 * k:128 * (k + 1), cs:cs + cw])
            wh = pt([128, 1024], F32R, tag="wh", bufs=4)[:, :cw]
            nc.vector.tensor_copy(wh, wkf)
            if nterms <= 2:
                return [wh]
            wl = pt([128, 1024], F32R, tag="wl", bufs=3)[:, :cw]
            nc.vector.scalar_tensor_tensor(out=wl, in0=wkf, scalar=-1.0,
                                           in1=wh.bitcast(F32), op0=ALU.bypass,
                                           op1=ALU.subtract)
            return [wh, wl]

        def mm_terms(psum, wterms, aterms, dsl, start, stop):
            """Accumulate the configured term products into psum."""
            if len(wterms) == 1 and len(aterms) == 1:
                pairs = [(wterms[0], aterms[0])]
            elif len(wterms) == 1:
                pairs = [(wterms[0], aterms[0]), (wterms[0], aterms[1])]
            else:
                pairs = [(wterms[0], aterms[0]), (wterms[0], aterms[1]),
                         (wterms[1], aterms[0])]
            last = len(pairs) - 1
            for i, (w, a) in enumerate(pairs):
                nc.tensor.matmul(psum, w[:, dsl], a,
                                 start=(start and i == 0),
                                 stop=(stop and i == last))

        def psum512():
            return ps.tile([128, MT], F32, tag="mm", name="p")

        # ---------------- megatile loop ----------------
        for t in range(nT):
            ts = slice(t * MT, (t + 1) * MT)

            # ---- stage 1: h1 = relu(sc1 * (x @ Wc) + bi1) ----
            ps1 = [psum512() for _ in range(8)]
            for k in range(24):
                i, kk = divmod(k, 8)
                xk = pt([128, MT], F32, tag="xk", bufs=3)
                nc.sync.dma_start(out=xk, in_=xs[i][128 * kk:128 * (kk + 1), ts])
                aterms = split_act(xk, terms[0])
                wterms = load_w(Wc[i], kk, 0, 1024, terms[0])
                for d in range(8):
                    mm_terms(ps1[d], wterms, aterms,
                             slice(128 * d, 128 * (d + 1)),
                             start=(k == 0), stop=(k == 23))
            h1 = pt([128, 8, MT], F32, tag="h1", bufs=2)
            for d in range(8):
                nc.scalar.activation(h1[:, d, :], ps1[d], AF.Relu,
                                     bias=bi1_sb[:, d:d + 1],
                                     scale=sc1_sb[:, d:d + 1])

            # ---- W2: h2 = relu(sc2 * (h1 @ W2) + bi2) ----
            ps2 = [psum512() for _ in range(8)]
            for k in range(8):
                aterms = split_act(h1[:, k, :], terms[1])
                wterms = load_w(W2d, k, 0, 1024, terms[1])
                for d in range(8):
                    mm_terms(ps2[d], wterms, aterms,
                             slice(128 * d, 128 * (d + 1)),
                             start=(k == 0), stop=(k == 7))
            h2 = pt([128, 8, MT], F32, tag="h2", bufs=2)
            for d in range(8):
                nc.scalar.activation(h2[:, d, :], ps2[d], AF.Relu,
                                     bias=bi2_sb[:, d:d + 1],
                                     scale=sc2_sb[:, d:d + 1])

            # ---- router/S fold: zz[c] = h2_chunk^T @ AB  (fp32 exact) ----
            zzs = []
            for c in range(4):
                cs_ = slice(128 * c, 128 * (c + 1))
                psR = ps.tile([128, 48], F32, tag="mm", name="psR")
                for k in range(8):
                    nc.tensor.matmul(psR, h2[:, k, cs_], AB_sb[:, k, :],
                                     start=(k == 0), stop=(k == 7))
                zz = pt([128, 48], F32, tag="zz", bufs=4)
                nc.scalar.copy(zz, psR)
                zzs.append(zz)

            # ---- W3 -> z3 (+b3) for LN stats only ----
            if terms[2] == 1:
                h2r = pt([128, 8, MT], F32R, tag="h2r", bufs=2)
                for k in range(8):
                    nc.scalar.copy(h2r[:, k, :], h2[:, k, :])
            stat_parts = []
            for dg in range(2):
                ps3 = [psum512() for _ in range(6)]
                for k in range(8):
                    if terms[2] == 1:
                        aterms = [h2r[:, k, :]]
                    else:
                        aterms = split_act(h2[:, k, :], terms[2])
                    wterms = load_w(W3d, k, 768 * dg, 768, terms[2])
                    for d in range(6):
                        mm_terms(ps3[d], wterms, aterms,
                                 slice(128 * d, 128 * (d + 1)),
                                 start=(k == 0), stop=(k == 7))
                psum_s = ps.tile([1, MT], F32, tag="mm", name="psum_s")
                psum_q = ps.tile([1, MT], F32, tag="mm", name="psum_q")
                for d in range(6):
                    dd = 6 * dg + d
                    t3 = pt([128, MT], F32R, tag="t3", bufs=3)
                    nc.scalar.activation(t3, ps3[d], AF.Identity,
                                         bias=b3_sb[:, dd:dd + 1], scale=1.0)
                    t3q = pt([128, MT], F32R, tag="t3q", bufs=3)
                    nc.scalar.activation(t3q, ps3[d], AF.Square,
                                         bias=b3_sb[:, dd:dd + 1], scale=1.0)
                    nc.tensor.matmul(psum_s, ones_r, t3,
                                     start=(d == 0), stop=(d == 5))
                    nc.tensor.matmul(psum_q, ones_r, t3q,
                                     start=(d == 0), stop=(d == 5))
                sA = pt([1, MT], F32, tag="st", bufs=8)
                nc.scalar.copy(sA, psum_s)
                qA = pt([1, MT], F32, tag="st", bufs=8)
                nc.scalar.copy(qA, psum_q)
                stat_parts.append((sA, qA))

            # ---- LN stats: r = rsqrt(var+eps), mr = mean * r ----
            ssum = pt([1, MT], F32, tag="st", bufs=8)
            nc.vector.tensor_add(ssum, stat_parts[0][0], stat_parts[1][0])
            qsum = pt([1, MT], F32, tag="st", bufs=8)
            nc.vector.tensor_add(qsum, stat_parts[0][1], stat_parts[1][1])
            mean = pt([1, MT], F32, tag="st", bufs=8)
            nc.scalar.activation(mean, ssum, AF.Identity, scale=1.0 / D3)
            msq = pt([1, MT], F32, tag="st", bufs=8)
            nc.scalar.activation(msq, ssum, AF.Square, scale=1.0 / D3)
            e2p = pt([1, MT], F32, tag="st", bufs=8)
            nc.scalar.activation(e2p, qsum, AF.Copy, bias=EPS, scale=1.0 / D3)
            veps = pt([1, MT], F32, tag="st", bufs=8)
            nc.vector.scalar_tensor_tensor(out=veps, in0=msq, scalar=-1.0,
                                           in1=e2p, op0=ALU.mult, op1=ALU.add)
            sdev = pt([1, MT], F32, tag="st", bufs=8)
            nc.scalar.activation(sdev, veps, AF.Sqrt)
            r_sb = pt([1, MT], F32, tag="rsb", bufs=2)
            nc.vector.reciprocal(r_sb, sdev)
            mr_sb = pt([1, MT], F32, tag="mrsb", bufs=2)
            nc.vector.tensor_mul(mr_sb, mean, r_sb)
            # transpose (r, mr) to token-major columns via a DRAM bounce
            rscr = pt([2, MT], F32, tag="rscr", bufs=2, space="DRAM")
            nc.sync.dma_start(out=rscr[0:1, :], in_=r_sb)
            nc.sync.dma_start(out=rscr[1:2, :], in_=mr_sb)
            rm_all = pt([128, 2, 4], F32, tag="rm", bufs=2)
            for q in range(2):
                nc.sync.dma_start(
                    out=rm_all[:, q, :],
                    in_=rscr[q:q + 1, :].rearrange("q (c p) -> p (q c)", p=128))

            if skip_tail:
                for c in range(4):
                    nc.scalar.copy(out_tm[:, 4 * t + c:4 * t + c + 1],
                                   rm_all[:, 0, c:c + 1])
                continue

            # ---- per 128-token chunk: affine + top-2 + combine ----
            for c in range(4):
                cs_ = slice(128 * c, 128 * (c + 1))
                rcol = rm_all[:, 0, c:c + 1]
                mrcol = rm_all[:, 1, c:c + 1]
                zz = zzs[c]
                L0 = pt([128, 48], F32, tag="L0", bufs=2)
                nc.vector.tensor_add(L0, zz, CA_bc)
                L1 = pt([128, 48], F32, tag="L1", bufs=2)
                nc.vector.tensor_scalar(out=L1, in0=L0, scalar1=rcol,
                                        scalar2=None, op0=ALU.mult)
                L = pt([128, 48], F32, tag="L2", bufs=2)
                nc.vector.scalar_tensor_tensor(out=L, in0=Dn_bc, scalar=mrcol,
                                               in1=L1, op0=ALU.mult, op1=ALU.add)
                nc.vector.tensor_add(L, L, Eb_bc)
                acc = c0_bc
                for j in range(3):
                    lg = L[:, 16 * j:16 * j + 8]
                    Sv = L[:, 16 * j + 8:16 * j + 16]
                    max1 = pt([128, 1], F32, tag="mx1", bufs=2)
                    nc.vector.reduce_max(max1, lg, axis=AX.X)
                    is1 = pt([128, 8], F32, tag="is1", bufs=2)
                    nc.vector.tensor_scalar(out=is1, in0=lg, scalar1=max1,
                                            scalar2=None, op0=ALU.is_equal)
                    l2 = pt([128, 8], F32, tag="l2", bufs=2)
                    nc.vector.scalar_tensor_tensor(out=l2, in0=is1,
                                                   scalar=NEG_BIG, in1=lg,
                                                   op0=ALU.mult, op1=ALU.add)
                    max2 = pt([128, 1], F32, tag="mx2", bufs=2)
                    nc.vector.reduce_max(max2, l2, axis=AX.X)
                    is2 = pt([128, 8], F32, tag="is2", bufs=2)
                    nc.vector.tensor_scalar(out=is2, in0=l2, scalar1=max2,
                                            scalar2=None, op0=ALU.is_equal)
                    dd12 = pt([128, 1], F32, tag="dd12", bufs=2)
                    nc.vector.tensor_sub(dd12, max2, max1)
                    s2 = pt([128, 1], F32, tag="s2", bufs=2)
                    nc.scalar.activation(s2, dd12, AF.Sigmoid)
                    s1 = pt([128, 1], F32, tag="s1", bufs=2)
                    nc.scalar.activation(s1, dd12, AF.Sigmoid, scale=-1.0)
                    Wm = pt([128, 8], F32, tag="Wm", bufs=2)
                    nc.vector.tensor_scalar(out=Wm, in0=is1, scalar1=s1,
                                            scalar2=None, op0=ALU.mult)
                    nc.vector.scalar_tensor_tensor(out=Wm, in0=is2, scalar=s2,
                                                   in1=Wm, op0=ALU.mult,
                                                   op1=ALU.add)
                    Pp = pt([128, 8], F32, tag="Pp", bufs=2)
                    accn = pt([128, 1], F32, tag="acc", bufs=4)
                    if use_ttr:
                        nc.vector.tensor_tensor_reduce(
                            out=Pp, in0=Wm, in1=Sv, scale=1.0, scalar=acc,
                            op0=ALU.mult, op1=ALU.add, accum_out=accn)
                    else:
                        nc.vector.tensor_mul(Pp, Wm, Sv)
                        rsum = pt([128, 1], F32, tag="rsum", bufs=2)
                        nc.vector.reduce_sum(rsum, Pp, axis=AX.X)
                        nc.vector.tensor_add(accn, rsum, acc)
                    acc = accn
                nc.scalar.copy(out_tm[:, 4 * t + c:4 * t + c + 1], acc)

        nc.sync.dma_start(out=outd[:, :], in_=out_tm)

    nc.compile()
    return nc


def _pack_vec(v, nch):
    return np.ascontiguousarray(
        np.asarray(v, np.float64).astype(np.float32).reshape(nch, 128).T)


def prepare_maps(inputs):
    """Host-side fp64 folds + sharding. Returns per-core input maps."""
    f32 = np.float32
    g = lambda k: np.asarray(inputs[k], np.float64)
    k64 = 1.0 / np.sqrt(1.0 + np.float64(EPS))

    scf = g("bng") * k64
    wfr = g("Wf") @ (scf * g("Wr")[:, 0])                       # [1536]
    c0 = float((g("bf") * scf + g("bnb")) @ g("Wr")[:, 0] + g("br")[0])
    Wcs = [np.ascontiguousarray(
        (g(f"Wp{i+1}") @ g("W1")[512 * i:512 * (i + 1), :]).astype(f32))
        for i in range(3)]
    b1f = g("b1") + sum(g(f"bp{i+1}") @ g("W1")[512 * i:512 * (i + 1), :]
                        for i in range(3))
    g1, g2 = g("g1"), g("g2")
    lng, lnb = g("lng"), g("lnb")
    Wg, W3, b3 = g("Wg"), g("W3"), g("b3")
    We, bexp, bg = g("We"), g("bexp"), g("bg")

    AB = np.zeros((D3 // 512 * 0 + 1024, 48), np.float64)       # [1024, 48]
    CA = np.zeros(48, np.float64)
    Dn = np.zeros(48, np.float64)
    Eb = np.zeros(48, np.float64)
    for j in range(3):
        sl = slice(512 * j, 512 * (j + 1))
        Vj = (We @ wfr[sl]).T                                   # [512, 8]
        W3j = W3[:, sl] * lng[None, sl]                         # [1024, 512]
        AB[:, 16 * j:16 * j + 8] = W3j @ Wg
        AB[:, 16 * j + 8:16 * j + 16] = W3j @ Vj
        bl = b3[sl] * lng[sl]
        CA[16 * j:16 * j + 8] = bl @ Wg
        CA[16 * j + 8:16 * j + 16] = bl @ Vj
        Dn[16 * j:16 * j + 8] = -(lng[sl] @ Wg)
        Dn[16 * j + 8:16 * j + 16] = -(lng[sl] @ Vj)
        Eb[16 * j:16 * j + 8] = lnb[sl] @ Wg + bg
        Eb[16 * j + 8:16 * j + 16] = lnb[sl] @ Vj + bexp @ wfr[sl]

    consts = {
        "sc1": _pack_vec(g1 * k64, 8),
        "bi1": _pack_vec(b1f * g1 * k64 + g("be1"), 8),
        "sc2": _pack_vec(g2 * k64, 8),
        "bi2": _pack_vec(g("b2") * g2 * k64 + g("be2"), 8),
        "b3v": _pack_vec(b3, 12),
        "ABr": np.ascontiguousarray(
            AB.astype(f32).reshape(8, 128, 48).transpose(1, 0, 2)),
        "CAv": CA.astype(f32).reshape(1, 48),
        "Dnv": Dn.astype(f32).reshape(1, 48),
        "Ebv": Eb.astype(f32).reshape(1, 48),
        "c0v": np.full((1, 1), c0, f32),
        "ones_col": np.ones((128, 1), f32),
        "W2f": np.ascontiguousarray(np.asarray(inputs["W2"], f32)),
        "W3f": np.ascontiguousarray(np.asarray(inputs["W3"], f32)),
    }
    for i in range(3):
        consts[f"Wc{i+1}"] = Wcs[i]
    xts = [np.ascontiguousarray(np.asarray(inputs[f"x{i+1}"], f32).T)
           for i in range(3)]
    in_maps = []
    for c in range(N_CORES):
        m = dict(consts)
        sl = slice(c * TOK_CORE, (c + 1) * TOK_CORE)
        for i in range(3):
            m[f"x{i+1}t"] = np.ascontiguousarray(xts[i][:, sl])
        in_maps.append(m)
    return in_maps


def run(inputs, trace=False, terms=TERMS):
    key = terms
    if key not in _PROGRAM_CACHE:
        _PROGRAM_CACHE[key] = build_program(terms=terms)
    nc = _PROGRAM_CACHE[key]
    in_maps = prepare_maps(inputs)
    res = run_bass_kernel_spmd(nc, in_maps, list(range(N_CORES)), trace=trace)
    rows = []
    for c in range(N_CORES):
        o = np.asarray(res.results[c]["out"])          # [128, 16]
        rows.append(np.ascontiguousarray(o.T).reshape(-1))
    out = np.concatenate(rows).reshape(B, 1).astype(np.float32)
    return out, res


def kernel(**inputs):
    out, _ = run(inputs, trace=False)
    return out


# revision 23
# speedup vs baseline: 3.2487x; 1.3872x over previous
"""Trainium2 Bass kernel v2 for nn_CombinedMLPMoEModel (moe_routing).

Strategy (8 NeuronCores, pure data parallel on the batch):
 - Host: shard batch 16384 -> 8 x 2048 tokens, pre-transpose x1/x2/x3 to
   feature-major [Din, tok]; replicate (folded) weights.
 - Algebraic folds done on the host in fp64:
     * proj+W1:  h1pre = x @ (Wp_i @ W1_i) summed over i  (MAC-neutral,
       removes the intermediate `comb` activation entirely)
     * the tail (concat(o) @ Wf -> bn -> @ Wr) is linear in the MoE
       outputs o_j, so each expert's whole [tok,512] output folds to a
       scalar: S_j[:,e] = m_j @ (We_e @ wfr_j) + bexp_e . wfr_j  where
       wfr = Wf @ (bng*k*Wr).  The dense-expert compute, output
       transposes, bias matmuls and final fold all collapse into a tiny
       [512->8] fp32 matmul per part.
     * LayerNorm folds into those router/S matmuls: with
       A_j=(W3_j*lng_j)@Wg, B_j=(W3_j*lng_j)@V_j computed from h2
       directly, logits/S are affine in (r, mu*r): the normalized m is
       never materialized, and W3's matmul error never touches routing.
 - Precision: chain matmuls run fp32r (1 cyc/row).  TERMS controls a
   hi/lo activation/weight split per stage (1=single pass, 2=act split,
   3=act+weight split ~= exact) to keep top-2 routing flips rare; the
   router/S matmuls and LN-stat affine math are fp32-exact given h2.
 - Routing: top-2 of 8 via masked reduce_max, softmax pair via Sigmoid,
   combined with the folded per-expert scalars in one
   tensor_tensor_reduce per part.
"""

import numpy as np
from contextlib import ExitStack

import concourse.bass as bass
from concourse import bacc
import concourse.mybir as mybir
import concourse.tile as tile
from concourse.bass_utils import run_bass_kernel_spmd

F32 = mybir.dt.float32
F32R = mybir.dt.float32r
F8E4 = mybir.dt.float8e4
F8E5 = mybir.dt.float8e5
DR = mybir.MatmulPerfMode.DoubleRow
AF = mybir.ActivationFunctionType
ALU = mybir.AluOpType
AX = mybir.AxisListType

N_CORES = 8
B = 16384
DIN = 1024
D = 512
H = 1024
D3 = 3 * D          # 1536
E = 8
TOK_CORE = B // N_CORES   # 2048
MT = 512                  # megatile tokens
T = TOK_CORE // MT        # 4
EPS = 1e-5
NEG_BIG = -1.0e30

# per-stage fp32r term count: (stage1 x@Wc, W2, W3)
TERMS = (2, 2, 1)

_PROGRAM_CACHE = {}


def build_program(terms=TERMS, n_tok=TOK_CORE, use_ttr=False, skip_tail=False):
    nc = bacc.Bacc(None, target_bir_lowering=False)
    nT = n_tok // MT

    # ---------------- DRAM I/O ----------------
    xs = [nc.dram_tensor(f"x{i+1}t", [DIN, n_tok], F32, kind="ExternalInput")
          for i in range(3)]
    Wc = [nc.dram_tensor(f"Wc{i+1}", [DIN, H], F32, kind="ExternalInput")
          for i in range(3)]
    W2d = nc.dram_tensor("W2f", [H, H], F32, kind="ExternalInput")
    W3d = nc.dram_tensor("W3f", [H, D3], F32, kind="ExternalInput")
    sc1d = nc.dram_tensor("sc1", [128, 8], F32, kind="ExternalInput")
    bi1d = nc.dram_tensor("bi1", [128, 8], F32, kind="ExternalInput")
    sc2d = nc.dram_tensor("sc2", [128, 8], F32, kind="ExternalInput")
    bi2d = nc.dram_tensor("bi2", [128, 8], F32, kind="ExternalInput")
    b3d = nc.dram_tensor("b3v", [128, 12], F32, kind="ExternalInput")
    ABd = nc.dram_tensor("ABr", [128, 8, 48], F32, kind="ExternalInput")
    CAd = nc.dram_tensor("CAv", [1, 48], F32, kind="ExternalInput")
    Dnd = nc.dram_tensor("Dnv", [1, 48], F32, kind="ExternalInput")
    Ebd = nc.dram_tensor("Ebv", [1, 48], F32, kind="ExternalInput")
    c0d = nc.dram_tensor("c0v", [1, 1], F32, kind="ExternalInput")
    onesd = nc.dram_tensor("ones_col", [128, 1], F32, kind="ExternalInput")
    outd = nc.dram_tensor("out", [128, 4 * nT], F32, kind="ExternalOutput")

    with tile.TileContext(nc) as tc, ExitStack() as ctx:
        cp = ctx.enter_context(tc.tile_pool(name="consts", bufs=1))
        sp = ctx.enter_context(tc.tile_pool(name="work", bufs=1))
        ps = ctx.enter_context(tc.tile_pool(name="psum", bufs=8, space="PSUM"))

        def pt(shape, dtype, tag, bufs=2, space=None):
            return sp.tile(shape, dtype, tag=tag, bufs=bufs, name=tag,
                           space=space)

        # ---------------- resident constants ----------------
        AB_sb = cp.tile([128, 8, 48], F32, name="AB_sb")
        nc.sync.dma_start(out=AB_sb, in_=ABd[:, :, :])
        sc1_sb = cp.tile([128, 8], F32, name="sc1_sb")
        nc.sync.dma_start(out=sc1_sb, in_=sc1d[:, :])
        bi1_sb = cp.tile([128, 8], F32, name="bi1_sb")
        nc.sync.dma_start(out=bi1_sb, in_=bi1d[:, :])
        sc2_sb = cp.tile([128, 8], F32, name="sc2_sb")
        nc.sync.dma_start(out=sc2_sb, in_=sc2d[:, :])
        bi2_sb = cp.tile([128, 8], F32, name="bi2_sb")
        nc.sync.dma_start(out=bi2_sb, in_=bi2d[:, :])
        b3_sb = cp.tile([128, 12], F32, name="b3_sb")
        nc.sync.dma_start(out=b3_sb, in_=b3d[:, :])
        ones_r = cp.tile([128, 1], F32R, name="ones_r")
        nc.gpsimd.dma_start(out=ones_r, in_=onesd[:, :])

        def bcast48(dram, name):
            t = cp.tile([128, 48], F32, name=name)
            nc.gpsimd.dma_start(
                out=t,
                in_=bass.AP(tensor=dram[:, :].tensor, offset=0,
                            ap=[[0, 128], [1, 48]]))
            return t

        CA_bc = bcast48(CAd, "CA_bc")
        Dn_bc = bcast48(Dnd, "Dn_bc")
        Eb_bc = bcast48(Ebd, "Eb_bc")
        c0_bc = cp.tile([128, 1], F32, name="c0_bc")
        nc.gpsimd.dma_start(
            out=c0_bc,
            in_=bass.AP(tensor=c0d[:, :].tensor, offset=0, ap=[[0, 128], [1, 1]]))
        out_tm = cp.tile([128, 4 * nT], F32, name="out_tm")

        def split_act(src, nterms):
            """Return list of fp32r act term APs for one [128, 512] chunk.
            The BIR verifier requires fp32r matmul operands to come from a
            rounding producer, so even 1-term acts go through a copy."""
            ah = pt([128, MT], F32R, tag="ah", bufs=3)
            nc.scalar.copy(ah, src)
            if nterms == 1:
                return [ah]
            al = pt([128, MT], F32R, tag="al", bufs=3)
            nc.vector.scalar_tensor_tensor(out=al, in0=src, scalar=-1.0,
                                           in1=ah.bitcast(F32), op0=ALU.bypass,
                                           op1=ALU.subtract)
            return [ah, al]

        def load_w(dram, k, cs, cw, nterms):
            """Load weight chunk [128, cw] (k-chunk k, col slice cs..cs+cw);
            return list of fp32r weight term APs."""
            wkf = pt([128, 1024], F32, tag="wkf", bufs=3)[:, :cw]
            nc.sync.dma_start(out=wkf,
                              in_=dram[128 * k:128 * (k + 1), cs:cs + cw])
            wh = pt([128, 1024], F32R, tag="wh", bufs=4)[:, :cw]
            nc.vector.tensor_copy(wh, wkf)
            if nterms <= 2:
                return [wh]
            wl = pt([128, 1024], F32R, tag="wl", bufs=3)[:, :cw]
            nc.vector.scalar_tensor_tensor(out=wl, in0=wkf, scalar=-1.0,
                                           in1=wh.bitcast(F32), op0=ALU.bypass,
                                           op1=ALU.subtract)
            return [wh, wl]

        def mm_terms(psum, wterms, aterms, dsl, start, stop):
            """Accumulate the configured term products into psum."""
            if len(wterms) == 1 and len(aterms) == 1:
                pairs = [(wterms[0], aterms[0])]
            elif len(wterms) == 1:
                pairs = [(wterms[0], aterms[0]), (wterms[0], aterms[1])]
            else:
                pairs = [(wterms[0], aterms[0]), (wterms[0], aterms[1]),
                         (wterms[1], aterms[0])]
            last = len(pairs) - 1
            for i, (w, a) in enumerate(pairs):
                nc.tensor.matmul(psum, w[:, dsl], a,
                                 start=(start and i == 0),
                                 stop=(stop and i == last))

        def psum512():
            return ps.tile([128, MT], F32, tag="mm", name="p")

        # ---------------- megatile loop ----------------
        for t in range(nT):
            ts = slice(t * MT, (t + 1) * MT)

            # ---- stage 1: h1 = relu(sc1 * (x @ Wc) + bi1) ----
            # (Wc is shipped pre-scaled x16 so its fp8 copy lands in e4m3
            #  normal range; sc1 absorbs the 1/16 exactly.)
            ps1 = [psum512() for _ in range(8)]
            if terms[0] == 8:
                # 2-term with the al-correction in fp8 DoubleRow: k-chunk
                # pairs contract two planes per pass at 0.5 cyc/row.
                for P in range(12):
                    al8 = pt([128, 2, MT], F8E5, tag="al8", bufs=3)
                    w8 = pt([128, 2, 1024], F8E4, tag="w8", bufs=3)
                    ahs, whs = [], []
                    for i2 in range(2):
                        k = 2 * P + i2
                        i, kk = divmod(k, 8)
                        xk = pt([128, MT], F32, tag="xk", bufs=3)
                        nc.sync.dma_start(
                            out=xk, in_=xs[i][128 * kk:128 * (kk + 1), ts])
                        ah = pt([128, MT], F32R, tag="ah", bufs=3)
                        nc.scalar.copy(ah, xk)
                        ahs.append(ah)
                        nc.vector.scalar_tensor_tensor(
                            out=al8[:, i2, :], in0=xk, scalar=-1.0,
                            in1=ah.bitcast(F32), op0=ALU.bypass,
                            op1=ALU.subtract)
                        wkf = pt([128, 1024], F32, tag="wkf", bufs=3)
                        nc.sync.dma_start(
                            out=wkf, in_=Wc[i][128 * kk:128 * (kk + 1), :])
                        wh = pt([128, 1024], F32R, tag="wh", bufs=4)
                        nc.vector.tensor_copy(wh, wkf)
                        whs.append(wh)
                        nc.vector.tensor_copy(w8[:, i2, :], wkf)
                    for d in range(8):
                        dsl = slice(128 * d, 128 * (d + 1))
                        nc.tensor.matmul(ps1[d], whs[0][:, dsl], ahs[0],
                                         start=(P == 0), stop=False)
                        nc.tensor.matmul(ps1[d], whs[1][:, dsl], ahs[1],
                                         start=False, stop=False)
                        nc.tensor.matmul(ps1[d], w8[:, :, dsl], al8,
                                         perf_mode=DR,
                                         start=False, stop=(P == 11))
            else:
                for k in range(24):
                    i, kk = divmod(k, 8)
                    xk = pt([128, MT], F32, tag="xk", bufs=3)
                    nc.sync.dma_start(out=xk,
                                      in_=xs[i][128 * kk:128 * (kk + 1), ts])
                    aterms = split_act(xk, terms[0])
                    wterms = load_w(Wc[i], kk, 0, 1024, terms[0])
                    for d in range(8):
                        mm_terms(ps1[d], wterms, aterms,
                                 slice(128 * d, 128 * (d + 1)),
                                 start=(k == 0), stop=(k == 23))
            h1 = pt([128, 8, MT], F32, tag="h1", bufs=2)
            for d in range(8):
                nc.scalar.activation(h1[:, d, :], ps1[d], AF.Relu,
                                     bias=bi1_sb[:, d:d + 1],
                                     scale=sc1_sb[:, d:d + 1])

            # ---- W2: h2 = relu(sc2 * (h1 @ W2) + bi2) ----
            ps2 = [psum512() for _ in range(8)]
            for k in range(8):
                aterms = split_act(h1[:, k, :], terms[1])
                wterms = load_w(W2d, k, 0, 1024, terms[1])
                for d in range(8):
                    mm_terms(ps2[d], wterms, aterms,
                             slice(128 * d, 128 * (d + 1)),
                             start=(k == 0), stop=(k == 7))
            h2 = pt([128, 8, MT], F32, tag="h2", bufs=2)
            for d in range(8):
                nc.scalar.activation(h2[:, d, :], ps2[d], AF.Relu,
                                     bias=bi2_sb[:, d:d + 1],
                                     scale=sc2_sb[:, d:d + 1])

            # ---- router/S fold: zz[c] = h2_chunk^T @ AB  (fp32 exact) ----
            zzs = []
            for c in range(4):
                cs_ = slice(128 * c, 128 * (c + 1))
                psR = ps.tile([128, 48], F32, tag="mm", name="psR")
                for k in range(8):
                    nc.tensor.matmul(psR, h2[:, k, cs_], AB_sb[:, k, :],
                                     start=(k == 0), stop=(k == 7))
                zz = pt([128, 48], F32, tag="zz", bufs=4)
                nc.scalar.copy(zz, psR)
                zzs.append(zz)

            # ---- W3 -> z3 (+b3) for LN stats only ----
            if terms[2] == 1:
                h2r = pt([128, 8, MT], F32R, tag="h2r", bufs=2)
                for k in range(8):
                    nc.scalar.copy(h2r[:, k, :], h2[:, k, :])
            stat_parts = []
            for dg in range(2):
                ps3 = [psum512() for _ in range(6)]
                for k in range(8):
                    if terms[2] == 1:
                        aterms = [h2r[:, k, :]]
                    else:
                        aterms = split_act(h2[:, k, :], terms[2])
                    wterms = load_w(W3d, k, 768 * dg, 768, terms[2])
                    for d in range(6):
                        mm_terms(ps3[d], wterms, aterms,
                                 slice(128 * d, 128 * (d + 1)),
                                 start=(k == 0), stop=(k == 7))
                psum_s = ps.tile([1, MT], F32, tag="mm", name="psum_s")
                psum_q = ps.tile([1, MT], F32, tag="mm", name="psum_q")
                for d in range(6):
                    dd = 6 * dg + d
                    t3 = pt([128, MT], F32R, tag="t3", bufs=3)
                    nc.scalar.activation(t3, ps3[d], AF.Identity,
                                         bias=b3_sb[:, dd:dd + 1], scale=1.0)
                    t3q = pt([128, MT], F32R, tag="t3q", bufs=3)
                    nc.scalar.activation(t3q, ps3[d], AF.Square,
                                         bias=b3_sb[:, dd:dd + 1], scale=1.0)
                    nc.tensor.matmul(psum_s, ones_r, t3,
                                     start=(d == 0), stop=(d == 5))
                    nc.tensor.matmul(psum_q, ones_r, t3q,
                                     start=(d == 0), stop=(d == 5))
                sA = pt([1, MT], F32, tag="st", bufs=8)
                nc.scalar.copy(sA, psum_s)
                qA = pt([1, MT], F32, tag="st", bufs=8)
                nc.scalar.copy(qA, psum_q)
                stat_parts.append((sA, qA))

            # ---- LN stats: r = rsqrt(var+eps), mr = mean * r ----
            ssum = pt([1, MT], F32, tag="st", bufs=8)
            nc.vector.tensor_add(ssum, stat_parts[0][0], stat_parts[1][0])
            qsum = pt([1, MT], F32, tag="st", bufs=8)
            nc.vector.tensor_add(qsum, stat_parts[0][1], stat_parts[1][1])
            mean = pt([1, MT], F32, tag="st", bufs=8)
            nc.scalar.activation(mean, ssum, AF.Identity, scale=1.0 / D3)
            msq = pt([1, MT], F32, tag="st", bufs=8)
            nc.scalar.activation(msq, ssum, AF.Square, scale=1.0 / D3)
            e2p = pt([1, MT], F32, tag="st", bufs=8)
            nc.scalar.activation(e2p, qsum, AF.Copy, bias=EPS, scale=1.0 / D3)
            veps = pt([1, MT], F32, tag="st", bufs=8)
            nc.vector.scalar_tensor_tensor(out=veps, in0=msq, scalar=-1.0,
                                           in1=e2p, op0=ALU.mult, op1=ALU.add)
            sdev = pt([1, MT], F32, tag="st", bufs=8)
            nc.scalar.activation(sdev, veps, AF.Sqrt)
            r_sb = pt([1, MT], F32, tag="rsb", bufs=2)
            nc.vector.reciprocal(r_sb, sdev)
            mr_sb = pt([1, MT], F32, tag="mrsb", bufs=2)
            nc.vector.tensor_mul(mr_sb, mean, r_sb)
            # transpose (r, mr) to token-major columns via a DRAM bounce
            rscr = pt([2, MT], F32, tag="rscr", bufs=2, space="DRAM")
            nc.sync.dma_start(out=rscr[0:1, :], in_=r_sb)
            nc.sync.dma_start(out=rscr[1:2, :], in_=mr_sb)
            rm_all = pt([128, 2, 4], F32, tag="rm", bufs=2)
            for q in range(2):
                nc.sync.dma_start(
                    out=rm_all[:, q, :],
                    in_=rscr[q:q + 1, :].rearrange("q (c p) -> p (q c)", p=128))

            if skip_tail:
                for c in range(4):
                    nc.scalar.copy(out_tm[:, 4 * t + c:4 * t + c + 1],
                                   rm_all[:, 0, c:c + 1])
                continue

            # ---- per 128-token chunk: affine + top-2 + combine ----
            for c in range(4):
                cs_ = slice(128 * c, 128 * (c + 1))
                rcol = rm_all[:, 0, c:c + 1]
                mrcol = rm_all[:, 1, c:c + 1]
                zz = zzs[c]
                L0 = pt([128, 48], F32, tag="L0", bufs=2)
                nc.vector.tensor_add(L0, zz, CA_bc)
                L1 = pt([128, 48], F32, tag="L1", bufs=2)
                nc.vector.tensor_scalar(out=L1, in0=L0, scalar1=rcol,
                                        scalar2=None, op0=ALU.mult)
                L = pt([128, 48], F32, tag="L2", bufs=2)
                nc.vector.scalar_tensor_tensor(out=L, in0=Dn_bc, scalar=mrcol,
                                               in1=L1, op0=ALU.mult, op1=ALU.add)
                nc.vector.tensor_add(L, L, Eb_bc)
                acc = c0_bc
                for j in range(3):
                    lg = L[:, 16 * j:16 * j + 8]
                    Sv = L[:, 16 * j + 8:16 * j + 16]
                    max1 = pt([128, 1], F32, tag="mx1", bufs=2)
                    nc.vector.reduce_max(max1, lg, axis=AX.X)
                    is1 = pt([128, 8], F32, tag="is1", bufs=2)
                    nc.vector.tensor_scalar(out=is1, in0=lg, scalar1=max1,
                                            scalar2=None, op0=ALU.is_equal)
                    l2 = pt([128, 8], F32, tag="l2", bufs=2)
                    nc.vector.scalar_tensor_tensor(out=l2, in0=is1,
                                                   scalar=NEG_BIG, in1=lg,
                                                   op0=ALU.mult, op1=ALU.add)
                    max2 = pt([128, 1], F32, tag="mx2", bufs=2)
                    nc.vector.reduce_max(max2, l2, axis=AX.X)
                    is2 = pt([128, 8], F32, tag="is2", bufs=2)
                    nc.vector.tensor_scalar(out=is2, in0=l2, scalar1=max2,
                                            scalar2=None, op0=ALU.is_equal)
                    dd12 = pt([128, 1], F32, tag="dd12", bufs=2)
                    nc.vector.tensor_sub(dd12, max2, max1)
                    s2 = pt([128, 1], F32, tag="s2", bufs=2)
                    nc.scalar.activation(s2, dd12, AF.Sigmoid)
                    s1 = pt([128, 1], F32, tag="s1", bufs=2)
                    nc.scalar.activation(s1, dd12, AF.Sigmoid, scale=-1.0)
                    Wm = pt([128, 8], F32, tag="Wm", bufs=2)
                    nc.vector.tensor_scalar(out=Wm, in0=is1, scalar1=s1,
                                            scalar2=None, op0=ALU.mult)
                    nc.vector.scalar_tensor_tensor(out=Wm, in0=is2, scalar=s2,
                                                   in1=Wm, op0=ALU.mult,
                                                   op1=ALU.add)
                    Pp = pt([128, 8], F32, tag="Pp", bufs=2)
                    accn = pt([128, 1], F32, tag="acc", bufs=4)
                    if use_ttr:
                        nc.vector.tensor_tensor_reduce(
                            out=Pp, in0=Wm, in1=Sv, scale=1.0, scalar=acc,
                            op0=ALU.mult, op1=ALU.add, accum_out=accn)
                    else:
                        nc.vector.tensor_mul(Pp, Wm, Sv)
                        rsum = pt([128, 1], F32, tag="rsum", bufs=2)
                        nc.vector.reduce_sum(rsum, Pp, axis=AX.X)
                        nc.vector.tensor_add(accn, rsum, acc)
                    acc = accn
                nc.scalar.copy(out_tm[:, 4 * t + c:4 * t + c + 1], acc)

        nc.sync.dma_start(out=outd[:, :], in_=out_tm)

    nc.compile()
    return nc


def _pack_vec(v, nch):
    return np.ascontiguousarray(
        np.asarray(v, np.float64).astype(np.float32).reshape(nch, 128).T)


def prepare_maps(inputs):
    """Host-side fp64 folds + sharding. Returns per-core input maps."""
    f32 = np.float32
    g = lambda k: np.asarray(inputs[k], np.float64)
    k64 = 1.0 / np.sqrt(1.0 + np.float64(EPS))

    scf = g("bng") * k64
    wfr = g("Wf") @ (scf * g("Wr")[:, 0])                       # [1536]
    c0 = float((g("bf") * scf + g("bnb")) @ g("Wr")[:, 0] + g("br")[0])
    # x16 so the fp8(e4m3) copy of Wc lands in normal range; sc1 absorbs it
    Wcs = [np.ascontiguousarray(
        (16.0 * g(f"Wp{i+1}") @ g("W1")[512 * i:512 * (i + 1), :]).astype(f32))
        for i in range(3)]
    b1f = g("b1") + sum(g(f"bp{i+1}") @ g("W1")[512 * i:512 * (i + 1), :]
                        for i in range(3))
    g1, g2 = g("g1"), g("g2")
    lng, lnb = g("lng"), g("lnb")
    Wg, W3, b3 = g("Wg"), g("W3"), g("b3")
    We, bexp, bg = g("We"), g("bexp"), g("bg")

    AB = np.zeros((D3 // 512 * 0 + 1024, 48), np.float64)       # [1024, 48]
    CA = np.zeros(48, np.float64)
    Dn = np.zeros(48, np.float64)
    Eb = np.zeros(48, np.float64)
    for j in range(3):
        sl = slice(512 * j, 512 * (j + 1))
        Vj = (We @ wfr[sl]).T                                   # [512, 8]
        W3j = W3[:, sl] * lng[None, sl]                         # [1024, 512]
        AB[:, 16 * j:16 * j + 8] = W3j @ Wg
        AB[:, 16 * j + 8:16 * j + 16] = W3j @ Vj
        bl = b3[sl] * lng[sl]
        CA[16 * j:16 * j + 8] = bl @ Wg
        CA[16 * j + 8:16 * j + 16] = bl @ Vj
        Dn[16 * j:16 * j + 8] = -(lng[sl] @ Wg)
        Dn[16 * j + 8:16 * j + 16] = -(lng[sl] @ Vj)
        Eb[16 * j:16 * j + 8] = lnb[sl] @ Wg + bg
        Eb[16 * j + 8:16 * j + 16] = lnb[sl] @ Vj + bexp @ wfr[sl]

    consts = {
        "sc1": _pack_vec(g1 * k64 / 16.0, 8),
        "bi1": _pack_vec(b1f * g1 * k64 + g("be1"), 8),
        "sc2": _pack_vec(g2 * k64, 8),
        "bi2": _pack_vec(g("b2") * g2 * k64 + g("be2"), 8),
        "b3v": _pack_vec(b3, 12),
        "ABr": np.ascontiguousarray(
            AB.astype(f32).reshape(8, 128, 48).transpose(1, 0, 2)),
        "CAv": CA.astype(f32).reshape(1, 48),
        "Dnv": Dn.astype(f32).reshape(1, 48),
        "Ebv": Eb.astype(f32).reshape(1, 48),
        "c0v": np.full((1, 1), c0, f32),
        "ones_col": np.ones((128, 1), f32),
        "W2f": np.ascontiguousarray(np.asarray(inputs["W2"], f32)),
        "W3f": np.ascontiguousarray(np.asarray(inputs["W3"], f32)),
    }
    for i in range(3):
        consts[f"Wc{i+1}"] = Wcs[i]
    xts = [np.ascontiguousarray(np.asarray(inputs[f"x{i+1}"], f32).T)
           for i in range(3)]
    in_maps = []
    for c in range(N_CORES):
        m = dict(consts)
        sl = slice(c * TOK_CORE, (c + 1) * TOK_CORE)
        for i in range(3):
            m[f"x{i+1}t"] = np.ascontiguousarray(xts[i][:, sl])
        in_maps.append(m)
    return in_maps


def run(inputs, trace=False, terms=TERMS):
    key = terms
    if key not in _PROGRAM_CACHE:
        _PROGRAM_CACHE[key] = build_program(terms=terms)
    nc = _PROGRAM_CACHE[key]
    in_maps = prepare_maps(inputs)
    res = run_bass_kernel_spmd(nc, in_maps, list(range(N_CORES)), trace=trace)
    rows = []
    for c in range(N_CORES):
        o = np.asarray(res.results[c]["out"])          # [128, 16]
        rows.append(np.ascontiguousarray(o.T).reshape(-1))
    out = np.concatenate(rows).reshape(B, 1).astype(np.float32)
    return out, res


def kernel(**inputs):
    out, _ = run(inputs, trace=False)
    return out
